# revision 41
# baseline (speedup 1.0000x reference)
"""Dropless MoE FFN (router + top-2 dispatch + per-expert MLP + combine) on
8 Trainium2 NeuronCores.

Strategy (tensor parallelism over the FFN dim -- perfectly load balanced):
  - Router (softmax + top-2) runs on host in fp32 (~0.02% of FLOPs); the
    token dispatch is a host-side gather: the 8192 (token, expert) pairs
    are sorted by expert into one column-major activation matrix shared
    by all cores.
  - Each core owns a 512-wide slice of the FFN dim of ALL experts
    (column-parallel W1, row-parallel W2).  It computes, for every routed
    column t with expert e(t):
        y_partial[:, t] = w2_e[fslice, :]^T gelu(w1_e[:, fslice]^T x_t)
    This makes the per-core PE work exactly uniform (8192 columns each,
    zero padding), unlike expert-parallelism which pads every core to the
    most-loaded expert.
  - The F-dim partial outputs are summed across cores on the host (the
    host combine/scatter already exists); fp32 partials keep the math
    identical to a single long PSUM accumulation.

Device kernel layout per core:
  Token columns are processed in single-expert chunks of <=512 (PSUM bank
  width).  Expert-block boundaries are baked into the instruction stream
  at build time (compiled per routing signature, cached).  GEMM1 keeps
  tokens on the moving dim (4 f-tile positions x 8 kc accumulation),
  GELU runs PSUM->SBUF on ScalarE, GEMM2 contracts the local 512 f-rows
  (8 d-tile positions x 4 fk accumulation), PSUM->SBUF copies ride the
  otherwise-idle VectorE, and each chunk leaves as one strided DMA.
  DMA rings are purpose-split so slot-WAR pacing never stalls compute
  issue: sync carries every input in exact consumption order (unpaced;
  pool slot-WAR throttles it and the queues stay continuously fed),
  gpsimd carries the y writeout, scalar only GELUs, vector only casts.
  A short junk-matmul chain absorbs the PE clock ramp during the head.
"""

import sys

for _p in ("/opt/trn_rl_repo",):
    if _p not in sys.path:
        sys.path.insert(0, _p)

import numpy as np
import ml_dtypes

BF16 = ml_dtypes.bfloat16

D_MODEL = 1024
D_FFN = 4096
N_EXPERTS = 8
TOP_K = 2
N_CORES = 8
P = 128                 # SBUF/PSUM partitions
KC = D_MODEL // P       # 8 contraction chunks for GEMM1 / d-tiles for GEMM2
FL = D_FFN // N_CORES   # 512 FFN columns owned per core
FLC = FL // P           # 4 local f-tiles

_kernel_cache: dict[tuple, object] = {}


def _token_groups(n, cap=512):
    """Split n token columns into <=cap-wide PSUM-bank-sized groups,
    as equal as possible."""
    n_g = -(-n // cap)
    base, rem = divmod(n, n_g)
    return [base + (1 if g < rem else 0) for g in range(n_g)]


def _make_chunks(counts):
    """Single-expert chunks of <=512 columns covering the expert-sorted
    column order; chunk sizes are baked into the program.  The very last
    chunk is halved (>=224 keeps LDWEIGHTS hidden under the matmuls) so
    the post-PE output flush is short."""
    chunks = []
    off = 0
    live = [e for e in range(N_EXPERTS) if counts[e] > 0]
    for e in live:
        sizes = _token_groups(counts[e])
        if e == live[-1] and sizes[-1] >= 448:
            h = sizes[-1] // 2
            sizes = sizes[:-1] + [sizes[-1] - h, h]
        for s in sizes:
            chunks.append((e, off, s))
            off += s
    return tuple(chunks)


def _build(chunks):
    import concourse.bass as bass
    import concourse.mybir as mybir
    import concourse.tile as tile
    from concourse import bacc

    dt = mybir.dt
    AF = mybir.ActivationFunctionType
    CT = sum(s for _, _, s in chunks)
    n_ch = len(chunks)

    nc = bacc.Bacc("TRN2", target_bir_lowering=False, debug=False,
                   num_devices=N_CORES)
    # chunk-major packing: each chunk's KC*s block is contiguous per
    # partition, so chunk DMAs are 128 rows of ~6-8KB instead of 1024
    # strided rows of ~1KB (2x queue efficiency on 32MB of traffic)
    xt_d = nc.dram_tensor("xt", [P, KC * CT], dt.bfloat16,
                          kind="ExternalInput").ap()
    # fi-major so a 256KB fi-band is 128 contiguous 2KB rows (fast DMA)
    w1_d = nc.dram_tensor("w1", [N_EXPERTS, FLC, P, KC * P], dt.bfloat16,
                          kind="ExternalInput").ap()
    w2_d = nc.dram_tensor("w2", [N_EXPERTS, P, FLC, D_MODEL], dt.bfloat16,
                          kind="ExternalInput").ap()
    y_d = nc.dram_tensor("y", [P, KC * CT], dt.bfloat16,
                         kind="ExternalOutput").ap()

    with tile.TileContext(nc) as tc:
        with (
            tc.tile_pool(name="w1", bufs=4) as w1_pool,
            tc.tile_pool(name="w2", bufs=3) as w2_pool,
            tc.tile_pool(name="xt", bufs=6) as xt_pool,
            tc.tile_pool(name="w1h", bufs=4) as w1h_pool,
            tc.tile_pool(name="w2h", bufs=4) as w2h_pool,
            tc.tile_pool(name="xth", bufs=3) as xth_pool,
            tc.tile_pool(name="ht", bufs=4) as ht_pool,
            tc.tile_pool(name="yo", bufs=3) as y_pool,
            tc.tile_pool(name="warm", bufs=1) as warm_pool,
            tc.tile_pool(name="ps", bufs=8, space=bass.MemorySpace.PSUM) as ps_pool,
        ):
            # ---- PE p-state warmup: ~3us of junk matmuls absorb the
            # clock ramp while the head DMAs stream in (PE idle anyway)
            warm = warm_pool.tile([P, 512], dt.bfloat16, tag="warm",
                                  name="warm")
            nc.vector.memzero(warm[:])
            wps = ps_pool.tile([P, 512], dt.float32, tag="ps", name="warm_ps")
            for _ in range(14):
                nc.tensor.matmul(wps[:, :256], warm[:, :128],
                                 warm[:, 256:512], start=True, stop=True)
            # ---- pass 1: all DMAs, in consumption order.  Pool slot-WAR
            # paces each ring automatically; rings carry only DMAs (plus
            # vector copies) so pacing never blocks compute issue.
            w1_t, w2_t = {}, {}
            first_e = chunks[0][0]
            s0 = chunks[0][2]
            # Head streaming: every piece the PE consumes early is its own
            # tile so RAW deps are fine-grained, split along the axis the
            # compute loops walk: w1[e0] by fi-band (GEMM1's fi=0 pass
            # needs 256KB, not the whole 1MB), xt(c0) by kc-band, w2[e0]
            # by fk-band (chunk0's GEMM2 runs fk-outer, so it starts after
            # one 256KB band).  Emission order interleaves them to match
            # first-use order; everything rides the fast sync HWDGE ring.
            w1_fib = [w1h_pool.tile([P, KC, P], dt.bfloat16, tag="w1h",
                                    name=f"w1h_{fi}") for fi in range(FLC)]
            xt_head = []
            head_bands = [(0, 1), (1, 3), (4, 4)]
            w2_fkb = [w2h_pool.tile([P, 1, D_MODEL], dt.bfloat16, tag="w2h",
                                    name=f"w2h_{fk}") for fk in range(FLC)]
            nc.sync.dma_start(w1_fib[0][:], w1_d[first_e][0])
            for k0, kn in head_bands:
                xh = xth_pool.tile([P, kn, s0], dt.bfloat16, tag="xth",
                                   name=f"xth_{k0}")
                nc.sync.dma_start(xh[:],
                                  xt_d[:, k0 * s0:(k0 + kn) * s0])
                for i in range(kn):
                    xt_head.append((xh, i))
            nc.sync.dma_start(w1_fib[1][:], w1_d[first_e][1])
            nc.sync.dma_start(w2_fkb[0][:], w2_d[first_e][:, 0:1, :])
            nc.sync.dma_start(w1_fib[2][:], w1_d[first_e][2])
            nc.sync.dma_start(w2_fkb[1][:], w2_d[first_e][:, 1:2, :])
            nc.sync.dma_start(w1_fib[3][:], w1_d[first_e][3])
            for fk in range(2, FLC):
                nc.sync.dma_start(w2_fkb[fk][:],
                                  w2_d[first_e][:, fk:fk + 1, :])
            # the bands ARE expert e0's weight storage for all its chunks
            w1_t[first_e] = lambda kc, lo, hi, _w=w1_fib: \
                _w[lo // 128][:, kc, lo % 128:lo % 128 + (hi - lo)]
            w2_t[first_e] = lambda fk, lo, hi, _w=w2_fkb: \
                _w[fk][:, 0, lo:hi]

            # Remaining inputs: ONE sync-ring stream in exact consumption
            # order, unpaced -- pool slot-WAR throttles it naturally, the
            # queues stay continuously fed, and arrival order matches
            # need order (no serialize-then-burst).
            xt_t = [lambda kc, _x=xt_head: _x[kc][0][:, _x[kc][1], :]]
            for ci, (e, off, s) in enumerate(chunks):
                if ci == 0:
                    continue
                if e not in w1_t:
                    wt = w1_pool.tile([P, FLC, KC, P], dt.bfloat16,
                                      tag="w1", name=f"w1_{e}")
                    for fi in range(FLC):
                        nc.sync.dma_start(wt[:, fi, :, :], w1_d[e][fi])
                    w1_t[e] = lambda kc, lo, hi, _w=wt: \
                        _w[:, lo // 128, kc, lo % 128:lo % 128 + (hi - lo)]
                t = xt_pool.tile([P, KC, s], dt.bfloat16, tag="xt",
                                 name=f"xt_{ci}")
                nc.sync.dma_start(t[:], xt_d[:, KC * off:KC * (off + s)])
                xt_t.append(lambda kc, _t=t: _t[:, kc, :])
                if e not in w2_t:
                    # after this chunk's xt: w2[e] is consumed one GEMM1
                    # later than w1[e]/xt
                    w2t = w2_pool.tile([P, FLC, D_MODEL], dt.bfloat16,
                                       tag="w2", name=f"w2_{e}")
                    nc.sync.dma_start(w2t[:], w2_d[e])
                    w2_t[e] = lambda fk, lo, hi, _w=w2t: _w[:, fk, lo:hi]

            # ---- compute, chunk by chunk
            for ci, (e, off, s) in enumerate(chunks):
                xc = xt_t[ci]
                w1c, w2c = w1_t[e], w2_t[e]
                # GEMM1 + GELU: ht[fi*128+p, t] = gelu(sum_k w1[k, f] x[k, t])
                ht = ht_pool.tile([P, FLC, s], dt.bfloat16, tag="ht",
                                  name=f"ht_{ci}")
                for fi in range(FLC):
                    ps = ps_pool.tile([P, 512], dt.float32, tag="ps",
                                      name=f"ps1_{ci}_{fi}")
                    for kc in range(KC):
                        nc.tensor.matmul(ps[:, :s],
                                         w1c(kc, fi * P, (fi + 1) * P),
                                         xc(kc),
                                         start=(kc == 0), stop=(kc == KC - 1))
                    nc.scalar.activation(ht[:, fi, :], ps[:, :s],
                                         AF.Gelu_apprx_tanh)
                # GEMM2: y[dt*128+p, t] = sum_f w2[f, d] ht[f, t]  (local f)
                tailing = ci >= n_ch - 2
                if tailing:
                    # casts WAW-chain on a shared ysb tile (~700ns/link);
                    # four independent pair-tiles let the final cast land
                    # ~0.9us after the last matmul instead of ~2.5us
                    ypr = [y_pool.tile([P, 2, s], dt.bfloat16, tag="yo",
                                       name=f"yp_{ci}_{j}")
                           for j in range(KC // 2)]
                else:
                    ysb = y_pool.tile([P, KC, s], dt.bfloat16, tag="yo",
                                      name=f"y_{ci}")
                if ci == 0:
                    # fk-outer so GEMM2 starts once the first 256KB w2
                    # fk-band lands (head streaming); 8 PSUM banks in flight
                    psg = [ps_pool.tile([P, 512], dt.float32, tag="ps",
                                        name=f"ps2_0_{dtl}")
                           for dtl in range(KC)]
                    for fk in range(FLC):
                        for dtl in range(KC):
                            nc.tensor.matmul(psg[dtl][:, :s],
                                             w2c(fk, dtl * P, (dtl + 1) * P),
                                             ht[:, fk, :],
                                             start=(fk == 0),
                                             stop=(fk == FLC - 1))
                    for dtl in range(KC):
                        nc.vector.tensor_copy(ysb[:, dtl, :], psg[dtl][:, :s])
                    nc.gpsimd.dma_start(y_d[:, KC * off:KC * (off + s)], ysb[:])
                    continue
                for dtl in range(KC):
                    ps = ps_pool.tile([P, 512], dt.float32, tag="ps",
                                      name=f"ps2_{ci}_{dtl}")
                    for fk in range(FLC):
                        nc.tensor.matmul(ps[:, :s],
                                         w2c(fk, dtl * P, (dtl + 1) * P),
                                         ht[:, fk, :],
                                         start=(fk == 0), stop=(fk == FLC - 1))
                    if tailing:
                        if dtl % 2:
                            nc.scalar.copy(ypr[dtl // 2][:, 1, :], ps[:, :s])
                            # ship each pair as soon as its casts drain
                            nc.sync.dma_start(
                                y_d[:, KC * off + (dtl - 1) * s:
                                    KC * off + (dtl + 1) * s],
                                ypr[dtl // 2][:])
                        else:
                            nc.vector.tensor_copy(ypr[dtl // 2][:, 0, :],
                                                  ps[:, :s])
                    else:
                        nc.vector.tensor_copy(ysb[:, dtl, :], ps[:, :s])
                if tailing:
                    pass
                elif ci == n_ch - 2:
                    # inputs are done by now; sync is idle and fast
                    nc.sync.dma_start(y_d[:, KC * off:KC * (off + s)], ysb[:])
                else:
                    nc.gpsimd.dma_start(y_d[:, KC * off:KC * (off + s)], ysb[:])

    nc.compile()
    return nc


def _route(x, router_w):
    """Replicate the reference router math (jax on CPU, fp32)."""
    import jax
    import jax.numpy as jnp

    with jax.default_device(jax.devices("cpu")[0]):
        xt = jnp.asarray(np.asarray(x, np.float32)).reshape(-1, D_MODEL)
        logits = xt @ jnp.asarray(np.asarray(router_w, np.float32))
        probs = jax.nn.softmax(logits, axis=-1)
        top_p, top_i = jax.lax.top_k(probs, TOP_K)
    return np.asarray(top_p), np.asarray(top_i)


def _run(x, router_w, w1, w2, trace=False):
    from concourse import bass_utils

    x = np.asarray(x, np.float32)
    w1 = np.asarray(w1, np.float32)
    w2 = np.asarray(w2, np.float32)
    B, S, _ = x.shape
    T = B * S
    xt = x.reshape(T, D_MODEL)

    top_p, top_i = _route(x, router_w)

    idxs, wts, counts = [], [], []
    for e in range(N_EXPERTS):
        hit = top_i == e                       # [T, K]
        sel = hit.any(axis=1)
        idx = np.nonzero(sel)[0]
        w = (top_p * hit).sum(axis=1)[sel]     # combine weight per routed token
        idxs.append(idx)
        wts.append(w.astype(np.float32))
        counts.append(len(idx))

    chunks = _make_chunks(counts)
    CT = sum(s for _, _, s in chunks)
    nc = _kernel_cache.get(chunks)
    if nc is None:
        nc = _build(chunks)
        _kernel_cache[chunks] = nc

    # expert-sorted gathered activations, [P, KC, CT] (partition = d % 128)
    cols = np.concatenate([idxs[e] for e in range(N_EXPERTS) if counts[e]])
    xg = xt[cols]                                        # [CT, D]
    xtk = xg.T.reshape(KC, P, CT).transpose(1, 0, 2)   # [P, KC, CT]
    xtb = np.ascontiguousarray(np.concatenate(
        [xtk[:, :, o:o + s].reshape(P, KC * s) for _, o, s in chunks],
        axis=1)).astype(BF16)

    in_maps = []
    for c in range(N_CORES):
        cs = c * FL
        w1b = np.ascontiguousarray(
            w1[:, :, cs:cs + FL].reshape(N_EXPERTS, KC, P, FLC, P)
            .transpose(0, 3, 2, 1, 4)
            .reshape(N_EXPERTS, FLC, P, KC * P)).astype(BF16)
        w2b = np.ascontiguousarray(
            w2[:, cs:cs + FL, :].reshape(N_EXPERTS, FLC, P, D_MODEL)
            .transpose(0, 2, 1, 3)).astype(BF16)
        in_maps.append({"xt": xtb, "w1": w1b, "w2": w2b})

    res = bass_utils.run_bass_kernel_spmd(
        nc, in_maps, core_ids=list(range(N_CORES)), trace=trace)

    # host combine: sum the F-dim partials, then weighted scatter per expert
    ysum = np.zeros((P, KC * CT), np.float32)
    for c in range(N_CORES):
        ysum += np.asarray(res.results[c]["y"], np.float32)
    yfull = np.empty((D_MODEL, CT), np.float32)
    for _, o, s in chunks:
        yfull[:, o:o + s] = (ysum[:, KC * o:KC * (o + s)]
                             .reshape(P, KC, s).transpose(1, 0, 2)
                             .reshape(D_MODEL, s))

    out = np.zeros((T, D_MODEL), np.float32)
    off = 0
    for e in range(N_EXPERTS):
        n = counts[e]
        if n == 0:
            continue
        out[idxs[e]] += wts[e][:, None] * yfull[:, off:off + n].T
        off += n
    return out.reshape(B, S, D_MODEL), res


def kernel(**inputs):
    out, _ = _run(inputs["x"], inputs["router_w"], inputs["w1"], inputs["w2"])
    return out
